# revision 11
# baseline (speedup 1.0000x reference)
"""BetweennessRoPE Trainium2 kernel — fixed-table fp16 formulation.

Math (why no betweenness is computed on device):
  score = relu(1 - (path-direct)/max(direct,1e-6)) lies in [0,1] by the
  triangle inequality, so between in [0, 1/2046] and
  pos_adj = (between-0.5)*0.1 in [-0.05, -0.05+4.888e-5].  Hence for
  every position frac = 0.95 + delta with |delta| <= ~1.1e-4 (including
  the fp32 rounding of fl(s + pos_adj) at s ~ 2048).  The interpolated
  tables therefore differ from fixed-f tables
      C[s] = (1-f)*fcos[s-1] + f*fcos[s],  f = 0.95 + 0.05/2046
  by <= ~1.1e-4 * |fcos[s]-fcos[s-1]|, giving output error ~2e-4 of the
  output scale — far below the 2e-2 gate.  s=0 is exact (clip pins
  adj_pos to 0 and C[0]=fcos[0]).  So the kernel is a pure elementwise
  rotation with per-(s,k) constants:
      oe = xe*cc - xo*ss ;  oo = xo*cc + xe*ss.

Numerics: fp16 x / tables / products / outputs (DVE computes fp32
internally, rounds once on write) add ~1.5e-3 relative noise — still
~10x under the gate — and halve both DMA traffic and DVE cycles
(2x_1P packed mode needs 16-bit dense operands).

Layout: host de-interleaves even/odd features and converts to fp16.
Per slice [128, 2048]: partition p, col (e, t, k), s = 128t + p,
d = 2k + e.  Tables  t1 = [cc|-ss],  t2 = [ss|cc]  ([128, 2048] each).
Per slice, 3 VectorE TT ops (fp16 2x packed, FD=2048):
  pq[:, :2048] = x * t1        -> [xe*cc | -xo*ss]
  pq[:, 2048:] = x * t2        -> [xe*ss |  xo*cc]
  og           = sum over e    -> [oe | oo]   (one add, 4D views)
Per-slice DMA granularity (0.5 MiB) keeps startup short: the first mul
only waits for the t1 half of the table plus one slice.  GpSimd is
left idle on purpose: concurrent Q7 TT ops contend on the shared
DVE/POOL SBUF port (measured 2.4-2.9x DVE slowdown).  Input rides the
SyncE DMA queue; tables and output stores ride the ScalarE queue.
"""

import os
import numpy as np

B, S, H, D = 4, 2048, 16, 128
N = B * H
NCORES = 8
NPC = N // NCORES    # 8 slices per core
NT = S // 128        # 16
K2 = D // 2          # 64
HK = S // 2          # 1024 (cols per e-half)

_cache = {}


def _make_tables():
    base = (1.0 / (10000.0 ** (np.arange(0, D, 2, dtype=np.float32)
                               / np.float32(D)))).astype(np.float32)
    freqs = (np.arange(S, dtype=np.float32)[:, None]
             * base[None, :]).astype(np.float32)
    fcos = np.cos(freqs).astype(np.float32)
    fsin = np.sin(freqs).astype(np.float32)
    lo = np.maximum(np.arange(S) - 1, 0)
    f = 0.95 + 0.05 / 2046.0
    cc = ((1.0 - f) * fcos[lo].astype(np.float64)
          + f * fcos.astype(np.float64))
    ss = ((1.0 - f) * fsin[lo].astype(np.float64)
          + f * fsin.astype(np.float64))

    def blk(t):  # [S, 64] -> [128, NT*64], col (t, k)
        return t.reshape(NT, 128, K2).transpose(1, 0, 2).reshape(128, HK)

    t1 = np.concatenate([blk(cc), blk(-ss)], axis=1)       # [128, 2048]
    return np.ascontiguousarray(t1).astype(np.float16)


def _build_nc():
    import concourse.bacc as bacc
    import concourse.mybir as mybir
    from concourse.tile import TileContext

    f16 = mybir.dt.float16

    nc = bacc.Bacc()
    XC = nc.dram_tensor("XC", [NPC, 128, S], f16, kind="ExternalInput")
    OUT = nc.dram_tensor("OUT", [NPC, 128, S], f16, kind="ExternalOutput")
    CB = nc.dram_tensor("CB", [128, S], f16, kind="ExternalInput")

    # slices 0-1 run singly (compute starts after one 0.5 MiB load);
    # the rest run in pairs to amortize DVE per-op overhead.
    steps = [(0, 1), (1, 1), (2, 2), (4, 2), (6, 2)]

    with TileContext(nc) as tc:
        with (
            tc.tile_pool(name="const", bufs=1) as cpool,
            tc.tile_pool(name="xbuf", bufs=8) as xpool,
            tc.tile_pool(name="obuf", bufs=4) as opool,
            tc.tile_pool(name="pq", bufs=2) as wpool,
        ):
            tb = cpool.tile([128, 2 * S], f16, tag="tb", name="tb")
            nc.scalar.dma_start(tb[:, 0:S], CB[:, :])
            # t2 = [ss | cc] derived from t1 = [cc | -ss]
            nc.vector.tensor_scalar_mul(tb[:, S:S + HK], tb[:, HK:S], -1.0)
            nc.vector.tensor_copy(tb[:, S + HK:2 * S], tb[:, 0:HK])
            for n0, w in steps:
                cw = w * S
                xts = []
                for nl in range(w):
                    xt = xpool.tile([128, S], f16, tag="x",
                                    name=f"x{n0 + nl}")
                    nc.sync.dma_start(xt[:, :], XC[n0 + nl])
                    xts.append(xt)
                pq = wpool.tile([128, 2 * cw], f16, tag=f"PQ{w}",
                                name=f"PQ{n0}")
                og = opool.tile([128, cw], f16, tag=f"o{w}",
                                name=f"o{n0}")
                # pq cols per slice nl: [x*t1 | x*t2] = (m, e, c)
                for nl in range(w):
                    nc.vector.tensor_mul(pq[:, 2 * S * nl:2 * S * nl + S],
                                         xts[nl][:, :], tb[:, 0:S])
                    nc.vector.tensor_mul(pq[:, 2 * S * nl + S:2 * S * (nl + 1)],
                                         xts[nl][:, :], tb[:, S:2 * S])
                last_step = n0 + w == NPC
                for nl in range(w):
                    avn = (pq[:, 2 * S * nl:2 * S * (nl + 1)]
                           .rearrange("p (m e c) -> p m e c", m=2, e=2))
                    ogn = og[:, S * nl:S * (nl + 1)]
                    ovn = ogn.rearrange("p (m c) -> p m c", m=2)
                    if last_step and nl == w - 1:
                        # split the final add + store so the out stream
                        # drains while oo is still being computed
                        nc.vector.tensor_add(ovn[:, 0, :], avn[:, 0, 0, :],
                                             avn[:, 0, 1, :])
                        nc.scalar.dma_start(OUT[n0 + nl][:, 0:HK],
                                            ogn[:, 0:HK])
                        nc.vector.tensor_add(ovn[:, 1, :], avn[:, 1, 0, :],
                                             avn[:, 1, 1, :])
                        nc.scalar.dma_start(OUT[n0 + nl][:, HK:S],
                                            ogn[:, HK:S])
                    else:
                        nc.vector.tensor_add(ovn[:, :, :], avn[:, :, 0, :],
                                             avn[:, :, 1, :])
                        nc.scalar.dma_start(OUT[n0 + nl], ogn)
    nc.compile()
    return nc


def _get_built():
    if "nc" not in _cache:
        _cache["nc"] = _build_nc()
    return _cache["nc"]


def kernel(x, W, b):
    from concourse.bass_utils import run_bass_kernel_spmd

    assert x.shape == (B, S, H, D)
    xc = np.transpose(np.asarray(x, dtype=np.float32),
                      (0, 2, 1, 3)).reshape(N, S, D)
    # col (e, t, k) <- xc[n, 128t+p, 2k+e], fp16
    xs = np.ascontiguousarray(
        xc.reshape(N, NT, 128, K2, 2).transpose(0, 2, 4, 1, 3)
        .reshape(N, 128, S)).astype(np.float16)
    if "cb" not in _cache:
        _cache["cb"] = _make_tables()
    cbb = _cache["cb"]

    nc = _get_built()
    in_maps = []
    for c in range(NCORES):
        in_maps.append({
            "XC": np.ascontiguousarray(xs[NPC * c:NPC * (c + 1)]),
            "CB": cbb,
        })
    res = run_bass_kernel_spmd(nc, in_maps, core_ids=list(range(NCORES)))
    if res.exec_time_ns is not None:
        print(f"HW exec time: {res.exec_time_ns} ns")
    outs = np.concatenate([res.results[c]["OUT"] for c in range(NCORES)],
                          axis=0)                   # [N, 128, S]
    # og col = (m, t, k): s = 128t + p, d = 2k + m
    full = (outs.reshape(N, 128, 2, NT, K2).transpose(0, 3, 1, 4, 2)
            .reshape(N, S, D).astype(np.float32))
    full = full.reshape(B, H, S, D).transpose(0, 2, 1, 3)
    return np.ascontiguousarray(full)
